# revision 21
# baseline (speedup 1.0000x reference)
"""Trainium2 Bass kernel for nn_Attention_86646670230179 (eager MHA, f32 I/O).

Strategy (8 NeuronCores, tensor-parallel over heads, collective-free):
  - Each core owns 2 of the 16 heads (a 128-row slice of the internal dim).
  - v2: fully software-pipelined single instruction stream.
    * Host stages q/k/v as [128, B, NT, KT, 512] bf16 so each (batch, 512-col
      block) is one contiguous 1 MB DMA, issued in consumption order (kills
      the startup and batch-boundary DMA stalls of v1).
    * Projection work is split into small "units" (one 512-col psum group or
      one transpose) kept in a filler queue. During the ACT-bound attention
      inner loop (exp of [128,1024] per 128-key tile is the longest stage at
      ~1.1 us vs ~0.9 us of PE work), 1-3 filler steps are pumped per key
      tile, hiding the batch-1 projections and the per-block output
      projections inside the exp shadow.
    * ensure()/need-order draining guarantees a unit's instructions are fully
      emitted before any attention instruction that consumes its outputs
      (PE is in-order; emitting a consumer before its producer would
      deadlock the engine queues).
  - Numerics identical to v1: score scale folded into Wq/bq, exp without max
    subtraction (scores ~ N(0,1)), PV matmul with an appended ones-column
    giving row sums for free, softmax division deferred to a [1,512]
    reciprocal_approx_fast + gpsimd partition-broadcast + one multiply.
  - Each core applies its slice of the output projection (bf16 partials);
    the host sums the 8 partials (the all-reduce of the reference sharding)
    and adds (bv @ Wo + bo), which commutes with attention exactly because
    softmax rows sum to 1.
"""
import sys
from collections import deque
from contextlib import ExitStack

import numpy as np

sys.path.insert(0, "/opt/trn_rl_repo")

import ml_dtypes  # noqa: E402
import concourse.bass as bass  # noqa: E402
import concourse.mybir as mybir  # noqa: E402
import concourse.tile as tile  # noqa: E402
from concourse import bacc  # noqa: E402
from concourse.bass_utils import run_bass_kernel_spmd  # noqa: E402
from concourse.masks import make_identity  # noqa: E402

BF16 = mybir.dt.bfloat16
F32 = mybir.dt.float32
AF = mybir.ActivationFunctionType

NCORES = 8
B, L, E, H = 2, 2048, 1024, 16
S = L
D = E // H            # 64 head dim
R = B * L             # 4096 total rows
HC = H // NCORES      # 2 heads per core
EC = HC * D           # 128 channel slice per core
KT = E // 128         # 8 contraction tiles
NT = L // 512         # 4 512-wide row tiles per batch
ST = S // 128         # 16 key tiles per batch
STN = ST // NT        # 4 key tiles per 512-row block
DP1 = D + 1           # 65: head dim + ones column


def build_nc():
    nc = bacc.Bacc("TRN2", target_bir_lowering=False, num_devices=NCORES)

    qT = nc.declare_dram_parameter("qT", [128, B, NT, KT, 512], BF16, isOutput=False)
    kT = nc.declare_dram_parameter("kT", [128, B, NT, KT, 512], BF16, isOutput=False)
    vT = nc.declare_dram_parameter("vT", [128, B, NT, KT, 512], BF16, isOutput=False)
    wq = nc.declare_dram_parameter("wq", [128, KT * EC], BF16, isOutput=False)
    wk = nc.declare_dram_parameter("wk", [128, KT * EC], BF16, isOutput=False)
    wv = nc.declare_dram_parameter("wv", [128, KT * EC], BF16, isOutput=False)
    wo = nc.declare_dram_parameter("wo", [128, E], BF16, isOutput=False)
    bq = nc.declare_dram_parameter("bq", [EC, 1], F32, isOutput=False)
    bk = nc.declare_dram_parameter("bk", [EC, 1], F32, isOutput=False)
    outTp = nc.declare_dram_parameter("outTp", [E, R], BF16, isOutput=True)

    with tile.TileContext(nc) as tc, ExitStack() as ctx:
        consts = ctx.enter_context(tc.tile_pool(name="consts", bufs=1))
        xk_pool = ctx.enter_context(tc.tile_pool(name="xk", bufs=4))
        xv_pool = ctx.enter_context(tc.tile_pool(name="xv", bufs=4))
        xq_pool = ctx.enter_context(tc.tile_pool(name="xq", bufs=4))
        vpt_pool = ctx.enter_context(tc.tile_pool(name="vpt", bufs=2))
        exp_pool = ctx.enter_context(tc.tile_pool(name="expp", bufs=4))
        pou_pool = ctx.enter_context(tc.tile_pool(name="poup", bufs=3))
        ot_pool = ctx.enter_context(tc.tile_pool(name="otp", bufs=6))
        ov_pool = ctx.enter_context(tc.tile_pool(name="ovp", bufs=4))
        rc_pool = ctx.enter_context(tc.tile_pool(name="rcp", bufs=2))
        rcb_pool = ctx.enter_context(tc.tile_pool(name="rcbp", bufs=2))
        # PSUM banks: sc 2x[128,1024] (4) + pv 3x[128,512] (3) + pp 1 (1) = 8
        psum_sc = ctx.enter_context(tc.tile_pool(name="psc", bufs=2, space="PSUM"))
        psum_pv = ctx.enter_context(tc.tile_pool(name="ppv", bufs=3, space="PSUM"))
        psum_pp = ctx.enter_context(tc.tile_pool(name="ppp", bufs=1, space="PSUM"))

        # ---- weights + staged x chunks, DMA'd in consumption order
        wq_sb = consts.tile([128, KT, EC], BF16, tag="wq")
        wk_sb = consts.tile([128, KT, EC], BF16, tag="wk")
        wv_sb = consts.tile([128, KT, EC], BF16, tag="wv")
        wo_sb = consts.tile([128, KT, EC], BF16, tag="wo")
        bq_sb = consts.tile([EC, 1], F32, tag="bq")
        bk_sb = consts.tile([EC, 1], F32, tag="bk")
        nc.gpsimd.dma_start(bk_sb[:], bk[:])
        nc.gpsimd.dma_start(bq_sb[:], bq[:])

        xch = {}
        xsrc = {"k": (kT, xk_pool), "v": (vT, xv_pool), "q": (qT, xq_pool)}

        def stage_chunk(b, name, n, split=False):
            src, pool = xsrc[name]
            t = pool.tile([128, KT, 512], BF16, tag=f"x{name}",
                          name=f"x{name}_{b}_{n}")
            if split:
                half = KT // 2
                nc.sync.dma_start(t[:, 0:half], src[:, b, n, 0:half])
                nc.sync.dma_start(t[:, half:KT], src[:, b, n, half:KT])
            else:
                nc.sync.dma_start(t[:], src[:, b, n])
            xch[(b, name, n)] = t

        # single sync ring; ONLY the prefix-critical transfers go upfront.
        # All other chunk DMAs are paced through the filler queue (each unit
        # carries the issue for the unit two ahead) so they never compete
        # with the critical path for HBM bandwidth.
        nc.sync.dma_start(wk_sb[:], wk[:].rearrange("p (ko m) -> p ko m", m=EC))
        stage_chunk(0, "k", 0, split=True)
        nc.sync.dma_start(wq_sb[:], wq[:].rearrange("p (ko m) -> p ko m", m=EC))
        stage_chunk(0, "q", 0, split=True)
        nc.sync.dma_start(wv_sb[:], wv[:].rearrange("p (ko m) -> p ko m", m=EC))
        stage_chunk(0, "v", 0, split=True)
        nc.sync.dma_start(wo_sb[:], wo[:].rearrange("p (m o) -> p m o", o=EC))

        ident = consts.tile([128, 128], BF16, tag="ident")
        make_identity(nc, ident[:])
        wtp = psum_pp.tile([128, 128], BF16, tag="pp", name="warm")
        for _ in range(40):
            nc.tensor.transpose(wtp[:], ident[:], ident[:])

        # persistent activation tiles
        qpT = [[consts.tile([128, 512], BF16, tag=f"qpT{b}_{n}", name=f"qpT{b}_{n}")
                for n in range(NT)] for b in range(B)]
        kpT = [[consts.tile([128, 512], BF16, tag=f"kpT{b}_{n}", name=f"kpT{b}_{n}")
                for n in range(NT)] for b in range(B)]
        vp = [[consts.tile([128, STN, HC, DP1], BF16, tag=f"vp{b}_{n}",
                           name=f"vp{b}_{n}")
               for n in range(NT)] for b in range(B)]
        for b in range(B):
            for n in range(NT):
                for h in range(HC):
                    nc.vector.memset(vp[b][n][:, :, h, D], 1.0)

        # ---- filler machinery: (unit_key, step_fn) queues
        proj_fillers = deque()
        outproj_fillers = deque()
        steps_left = {}

        def make_proj_steps(name, b, n):
            state = {}
            w_sb = {"k": wk_sb, "q": wq_sb, "v": wv_sb}[name]
            steps = []

            for kt in range(KT):
                def mm(kt=kt):
                    if kt == 0:
                        state["ps"] = psum_pp.tile([128, 512], F32, tag="pp",
                                                   name=f"ps_{name}_{b}_{n}")
                    nc.tensor.matmul(
                        state["ps"][:],
                        lhsT=w_sb[:, kt, :],
                        rhs=xch[(b, name, n)][:, kt, :],
                        start=(kt == 0),
                        stop=(kt == KT - 1),
                    )
                steps.append(mm)

            if name in ("k", "q"):
                dst = (kpT if name == "k" else qpT)[b][n]
                bias_sb = bk_sb if name == "k" else bq_sb

                def fin():
                    nc.vector.tensor_tensor(
                        dst[:], state["ps"][:],
                        bias_sb[:].to_broadcast((EC, 512)), mybir.AluOpType.add,
                    )
                steps.append(fin)
            else:
                def cast():
                    state["vpt"] = vpt_pool.tile([128, 512], BF16, tag="vpt",
                                                 name=f"vpt_{b}_{n}")
                    nc.vector.tensor_copy(state["vpt"][:], state["ps"][:])
                steps.append(cast)
                for mb in range(STN):
                    def tr(mb=mb):
                        trp = psum_pp.tile([128, 128], BF16, tag="pp")
                        nc.tensor.transpose(
                            trp[:], state["vpt"][:, mb * 128:(mb + 1) * 128],
                            ident[:],
                        )
                        nc.vector.tensor_copy(
                            vp[b][n][:, mb, :, 0:D],
                            trp[:].rearrange("p (i d) -> p i d", i=HC),
                        )
                    steps.append(tr)
            return steps

        def run_unit(key):
            for f in make_proj_steps(*key):
                f()
            steps_left[key] = 0

        def dma_step(key):
            def f():
                name, b, n = key
                if (b, name, n) not in xch:
                    stage_chunk(b, name, n)
            return f

        def enqueue_all(keys):
            # each unit carries the DMA issue for the unit two ahead so
            # transfers stay ~one unit in front of their consumers without
            # competing with the prefix-critical DMAs
            for k2 in keys[:6]:
                dma_step(k2)()
            for i, key in enumerate(keys):
                steps = []
                if i + 6 < len(keys):
                    steps.append(dma_step(keys[i + 6]))
                steps += make_proj_steps(*key)
                steps_left[key] = len(steps)
                for f in steps:
                    proj_fillers.append((key, f))

        def ensure(key):
            while steps_left.get(key, 0) > 0:
                k2, f = proj_fillers.popleft()
                f()
                steps_left[k2] -= 1

        periods_left = [B * NT * ST]

        def pump(nmax):
            k = 0
            while k < nmax:
                if proj_fillers:
                    k2, f = proj_fillers.popleft()
                    f()
                    steps_left[k2] -= 1
                elif outproj_fillers:
                    outproj_fillers.popleft()()
                else:
                    break
                k += 1

        def drain_proj():
            while proj_fillers:
                k2, f = proj_fillers.popleft()
                f()
                steps_left[k2] -= 1

        drain_mode = [False]
        drain_flip = [0]

        def make_outproj(ot, rowbase, m):
            def f():
                if drain_mode[0] and drain_flip[0] % 2:
                    # in the final drain the scores psum pool is idle —
                    # alternate banks so matmul/cast/DMA of consecutive
                    # tiles pipeline instead of serializing on one bank
                    big = psum_sc.tile([128, 1024], F32, tag="sc",
                                       name=f"ptd_{rowbase}_{m}")
                    pt = big[:, 0:512]
                else:
                    pt = psum_pp.tile([128, 512], F32, tag="pp",
                                      name=f"pt_{rowbase}_{m}")[:]
                drain_flip[0] += 1
                nc.tensor.matmul(
                    pt, lhsT=wo_sb[:, m, :], rhs=ot[:], start=True, stop=True,
                )
                ov = ov_pool.tile([128, 512], BF16, tag="ov", name=f"ov_{rowbase}_{m}")
                nc.vector.tensor_copy(ov[:], pt)
                eng = nc.scalar if (drain_mode[0] and drain_flip[0] % 2 == 0) else nc.sync
                eng.dma_start(
                    outTp[m * 128:(m + 1) * 128, rowbase:rowbase + 512], ov[:]
                )
            return f

        def emit_norm(po, b, lt, tail=False):
            # pou copy is the sole reader of po so the psum bank frees after
            # one fast DVE op (the recip->broadcast->mul chain reads SBUF)
            ot = ot_pool.tile([128, 512], BF16, tag="ot")
            for h in range(HC):
                pou = pou_pool.tile([DP1, 512], F32, tag="pou")
                if h == 0 or tail:
                    nc.scalar.activation(pou[:], po[h][0:DP1, :], AF.Copy)
                else:
                    nc.vector.tensor_copy(pou[:], po[h][0:DP1, :])
                # stage the sums row on partition 0 before the custom DVE op
                # (cross-partition base is only proven for tensor_copy)
                sums = rc_pool.tile([1, 512], F32, tag="sums")
                nc.vector.tensor_copy(sums[:], pou[D:DP1, :])
                rc = rc_pool.tile([1, 512], F32, tag="rc")
                nc.vector.reciprocal_approx_fast(rc[:], sums[:])
                rcb = rcb_pool.tile([D, 512], F32, tag="rcb")
                nc.gpsimd.partition_broadcast(rcb[:], rc[:])
                nc.vector.tensor_mul(ot[h * D:(h + 1) * D, :], pou[0:D, :], rcb[:])
            rowbase = b * L + lt * 512
            for m in range(KT):
                outproj_fillers.append(make_outproj(ot, rowbase, m))

        def attention(b, lt):
            ensure(("q", b, lt))
            po = [psum_pv.tile([128, 512], F32, tag="pv", name=f"po{b}{lt}{h}")
                  for h in range(HC)]
            for st in range(ST):
                ensure(("k", b, st // STN))
                ps = psum_sc.tile([128, 1024], F32, tag="sc")
                for h in range(HC):
                    nc.tensor.matmul(
                        ps[:, h * 512:(h + 1) * 512],
                        lhsT=kpT[b][st // STN][h * D:(h + 1) * D,
                                               (st % STN) * 128:(st % STN + 1) * 128],
                        rhs=qpT[b][lt][h * D:(h + 1) * D, :],
                        start=True,
                        stop=True,
                        tile_position=(h * D, 0),
                    )
                ex = exp_pool.tile([128, 1024], BF16, tag="exp")
                nc.scalar.activation(ex[:], ps[:], AF.Exp)
                # fillers go in the exp shadow: PV(st) cannot start before
                # exp(st) finishes, so filler matmuls emitted here run in
                # time the PE would otherwise idle
                backlog = len(proj_fillers) + len(outproj_fillers)
                periods_left[0] -= 1
                n = max(1, -(-backlog // max(periods_left[0], 1)))
                if 12 <= periods_left[0] < 32:
                    n = max(n, 2)
                elif periods_left[0] < 12:
                    n = min(n, 1)
                pump(min(3, n))
                ensure(("v", b, st // STN))
                for h in range(HC):
                    nc.tensor.matmul(
                        po[h][0:DP1, :],
                        lhsT=vp[b][st // STN][:, st % STN, h, :],
                        rhs=ex[:, h * 512:(h + 1) * 512],
                        start=(st == 0),
                        stop=(st == ST - 1),
                    )
            emit_norm(po, b, lt, tail=(b == 1 and lt == NT - 1))

        # ---- emission: minimal batch-0 prefix inline (k00, q00), the v00
        # unit and everything else pipelined into the attention loops
        run_unit(("k", 0, 0))
        run_unit(("q", 0, 0))
        unit_order = [("v", 0, 0)]
        for n in range(1, NT):
            unit_order += [("k", 0, n), ("v", 0, n)]
        unit_order += [("q", 0, n) for n in range(1, NT)]
        for n in range(NT):
            unit_order += [("k", 1, n), ("v", 1, n)]
        unit_order += [("q", 1, n) for n in range(NT)]
        enqueue_all(unit_order)

        for lt in range(NT):
            attention(0, lt)
        drain_proj()
        for lt in range(NT):
            attention(1, lt)
        drain_mode[0] = True
        while outproj_fillers:
            outproj_fillers.popleft()()

    nc.compile()
    return nc


_NC_CACHE = {}


def _get_nc():
    if "nc" not in _NC_CACHE:
        _NC_CACHE["nc"] = build_nc()
    return _NC_CACHE["nc"]


def _prearrange(w):
    # [E, EC] -> [128, KT*EC] partition-major so the device DMA is contiguous
    bf = ml_dtypes.bfloat16
    return np.ascontiguousarray(
        w.reshape(KT, 128, EC).transpose(1, 0, 2).reshape(128, KT * EC)
    ).astype(bf)


def kernel(q, k, v, Wq, bq, Wk, bk, Wv, bv, Wo, bo, _trace=False, _tmpdir=None):
    bf = ml_dtypes.bfloat16
    scale = np.float32(1.0 / np.sqrt(D))  # 0.125, exact

    def _stage_x(x):
        # [B, L, E] -> [128, B, NT, KT, 512]: per-(batch, 512-block) chunks
        # are contiguous per partition for single-descriptor DMAs
        xt = np.asarray(x, np.float32).reshape(B, NT, 512, KT, 128)
        return np.ascontiguousarray(xt.transpose(4, 0, 1, 3, 2)).astype(bf)

    qTh = _stage_x(q)
    kTh = _stage_x(k)
    vTh = _stage_x(v)
    Wq = np.asarray(Wq, np.float32)
    Wk = np.asarray(Wk, np.float32)
    Wv = np.asarray(Wv, np.float32)
    Wo = np.asarray(Wo, np.float32)

    in_maps = []
    for c in range(NCORES):
        sl = slice(c * EC, (c + 1) * EC)
        in_maps.append({
            "qT": qTh,
            "kT": kTh,
            "vT": vTh,
            "wq": _prearrange(Wq[:, sl] * scale),
            "wk": _prearrange(Wk[:, sl]),
            "wv": _prearrange(Wv[:, sl]),
            "wo": np.ascontiguousarray(Wo[sl, :]).astype(bf),
            "bq": (np.asarray(bq, np.float32)[sl] * scale).reshape(EC, 1).copy(),
            "bk": np.asarray(bk, np.float32)[sl].reshape(EC, 1).copy(),
        })

    nc = _get_nc()
    res = run_bass_kernel_spmd(
        nc, in_maps, list(range(NCORES)), trace=_trace, tmpdir=_tmpdir
    )
    # sum the per-core partial outputs (the all-reduce of the TP sharding)
    acc = np.zeros((E, R), np.float32)
    for c in range(NCORES):
        acc += np.asarray(res.results[c]["outTp"], np.float32)
    out = np.ascontiguousarray(acc.T)  # [R, E]
    # bv passes through attention unchanged (softmax rows sum to 1):
    # out += bv @ Wo + bo
    host_bias = (
        np.asarray(bv, np.float64) @ np.asarray(Wo, np.float64)
        + np.asarray(bo, np.float64)
    ).astype(np.float32)
    out += host_bias[None, :]
    if _trace:
        return out.reshape(B, L, E), res
    return out.reshape(B, L, E)
